# revision 78
# baseline (speedup 1.0000x reference)
"""NetVLAD Trainium2 kernel (Bass/Tile), data-parallel over batch on 8 cores.

Math (per batch b):
    x_hat = x / ||x||_2(channel)                    (B, D, H*W), D=512, N=1200
    logits = conv_w @ x_hat                         (K, N), K=64
    a = softmax_K(logits)
    vlad[k,d] = sum_n a[k,n] * x_hat[d,n] - (sum_n a[k,n]) * c[k,d]
    vlad = l2norm_rows(vlad); out = l2norm(flatten(vlad))   # == vlad_rows/8

Device-side structure (v8, DMA-transpose):
  - x is staged host-side as bf16 padded to N=1280 and DMA'd twice per
    batch: once in natural d-major layout (3 n-range parts) for the logits
    matmuls, and once through the DMA TRANSPOSE XBAR (16x128 tiles) into
    xt[p, j, d] = x[d, 10p+j].  This removes every PE transpose and every
    per-chunk PSUM eviction of the old design.  Pad pixels (n >= 1200)
    live in partitions 120:128 of every chunk and are zero.
  - logits are computed k-major: lgT[64, n] = sum_d wt[d,k] x[d,n], with
    wt chunks stationary and 512-wide bf16 moving x slices (1 cyc/row),
    accumulating over the 4 d-chunks into PSUM [64, 1200].  One ACT copy
    evicts lgT to fp16, and a second (SBUF->SBUF) DMA transpose turns it
    into n-major lgn[p, j, k] with the same 10p+j pixel mapping, ready for
    the batched n-major softmax tail.
  - softmax tail unchanged in spirit: sinv = exp(-0.5 ln(ss)); lgsc =
    lgn * sinv; exp; den; arden = expt*rden; atp = arden*sinv (bf16).
  - aggregation: vl[k,d] += atp_j^T @ xt_j over 10 chunks (bf16, 512-wide
    moving).  asum comes from s1[p,k] = sum_j arden (DVE reduce over the
    real partitions) + a tiny ones-moving matmul reducing partitions.
  - ss: 10 Square/STT accum passes over xt[0:120] (the real pixels),
    split ACT/DVE; ss is memset to 1.0 so pad lanes stay finite.
  - PSUM: lgT [64, 2, 1536] (2 parities x 3 bank-aligned 512-col matmul
    dests) + vl + asum = 8 banks.  The only PSUM recycling is the lgT
    parity, reused every other batch - no per-chunk rotation, no
    starvation coupling.
  - Warm matmuls (dest: junk cols of the asum bank) absorb the x DMA part
    semaphores so each first range matmul carries only the lgT parity WAR
    (walrus S3_LW allows one sync wait per Matmult).
  - rsqrt as exp(-0.5*ln), single ACT table set, gpsimd for tiny ops and
    the output DMA, software pipeline: tail of b-1 and epilog of b-2 run
    interleaved with batch b's matmuls.
"""

import numpy as np

import concourse.bass as bass
import concourse.mybir as mybir
from concourse import bacc
import concourse.tile as tile
from concourse.bass_utils import run_bass_kernel_spmd
from concourse.masks import make_identity
from concourse.tile_rust import add_dep_helper

F32 = mybir.dt.float32
F16 = mybir.dt.float16
BF16 = mybir.dt.bfloat16
ALU = mybir.AluOpType
ACTF = mybir.ActivationFunctionType

P = 128
BPC = 8            # batches per core
D = 512
N = 1200
NP = 1280          # padded pixel count (XBAR needs free % 128 == 0)
K = 64
DCH = D // P       # 4 d-chunks
NJ = 10            # pixel chunks; xt[p, j, :] = x[:, 128j + p]
NJREAL = [P] * 9 + [48]   # real partitions per chunk (n < 1200)
NRANGES = [(0, 512), (512, 1024), (1024, 1200)]
LN_EIGHTH = float(np.log(0.125))

SQ_ENG = "v a v v a v a v a v".split()


def _emit(nc):
    x = nc.dram_tensor("x", (BPC, D, NP), BF16, kind="ExternalInput")
    wt = nc.dram_tensor("wt", (D, K), BF16, kind="ExternalInput")
    cent = nc.dram_tensor("cent", (K, D), F32, kind="ExternalInput")
    out = nc.dram_tensor("out", (BPC, K, D), F32, kind="ExternalOutput")

    with tile.TileContext(nc) as tc:
        with (
            tc.tile_pool(name="const", bufs=1) as const,
            tc.tile_pool(name="xnat", bufs=4) as xnat_pool,
            tc.tile_pool(name="xtsb", bufs=4) as xt_pool,
            tc.tile_pool(name="softmax", bufs=2) as sm_pool,
            tc.tile_pool(name="smalls", bufs=2) as smalls,
            tc.tile_pool(name="epilog", bufs=2) as ep_pool,
            tc.tile_pool(name="psum", bufs=1, space="PSUM") as psum,
        ):
            wt_sb = const.tile([P, DCH, K], BF16)
            nc.sync.dma_start(wt_sb, wt[:, :].rearrange("(a p) k -> p a k", p=P))
            cent_sb = const.tile([K, D], F32)
            nc.sync.dma_start(cent_sb, cent[:, :])
            ln8 = const.tile([K, 1], F32)
            nc.gpsimd.memset(ln8, LN_EIGHTH)
            onesf = const.tile([P, 2], F32)
            nc.gpsimd.memset(onesf, 1.0)
            identf = const.tile([K, K], F32)
            make_identity(nc, identf)
            identh = const.tile([K, K], F16)
            nc.vector.tensor_copy(identh, identf)
            # never-read junk outputs for square-accumulate passes
            sqj = const.tile([P, D], BF16)
            sqj2 = const.tile([P, D], BF16)
            sqj3 = const.tile([K, D], BF16)

            # PSUM (6 of 8 banks): k-major logits (3 bank-aligned 512-col
            # matmul dests, no parity - the eviction happens well before the
            # next batch's first logits matmul), the n-major transposed
            # logits (fp16, written by PE transposes, read by the DVE
            # prescale), vlad, asum (+junk cols for warms).
            lgT = psum.tile([K, 3, 512], F32)      # 3 banks
            lgnP = psum.tile([P, NJ, K], F16)      # 1 bank
            vl = psum.tile([K, D], F32)            # 1 bank
            asum = psum.tile([K, 4], F32)          # 1 bank
            nc.vector.memset(lgnP.bitcast(F32), 0.0)

            # Startup warms: absorb the wt DMA / onesf memset / identh copy
            # semaphores so no first real matmul carries more than one wait.
            w0 = nc.tensor.matmul(
                asum[0:2, 2:4], wt_sb[:, 3, 0:2], wt_sb[:, 3, 0:2],
                start=True, stop=True, skip_group_check=True,
            )
            w1 = nc.tensor.matmul(
                asum[0:2, 2:4], onesf[:, 0:2], onesf[:, 0:2],
                start=True, stop=True, skip_group_check=True,
            )
            add_dep_helper(w1.ins, w0.ins, sync=False, reason="warm chain")
            w2 = nc.tensor.matmul(
                lgnP[0:2, 0, 0:2], identh[:, 0:2], identh[:, 0:2],
                is_transpose=True, start=True, stop=True,
                skip_group_check=True,
            )
            add_dep_helper(w2.ins, w1.ins, sync=False, reason="warm chain")

            state = {}

            def tail_pieces(b):
                """Softmax tail of batch b (runs in-batch as fillers)."""

                def t0():  # ACT: sinv = exp(-0.5*ln(ss)) (fp16 for DVE 2x)
                    st = state[b]
                    lss = smalls.tile([P, NJ], F32, tag="lss")
                    nc.scalar.activation(lss, st["ss"], ACTF.Ln)
                    sinv = smalls.tile([P, NJ], F16, tag="sinv")
                    nc.scalar.activation(sinv, lss, ACTF.Exp, scale=-0.5)
                    st["sinv"] = sinv

                def t1():  # DVE: prescale logits (reads the fp16 PSUM bank)
                    st = state[b]
                    lgsc = sm_pool.tile([P, NJ, K], BF16, tag="lgsc")
                    nc.vector.tensor_tensor(
                        lgsc,
                        lgnP[:, :, :],
                        st["sinv"].unsqueeze(-1).to_broadcast((P, NJ, K)),
                        ALU.mult,
                    )
                    st["lgsc"] = lgsc

                def t2():  # ACT: one big exp
                    st = state[b]
                    expt = sm_pool.tile([P, NJ, K], BF16, tag="expt")
                    nc.scalar.activation(expt, st["lgsc"], ACTF.Exp)
                    st["expt"] = expt

                def t3():  # DVE: denominators
                    st = state[b]
                    den = smalls.tile([P, NJ], F32, tag="den")
                    nc.vector.tensor_reduce(
                        den, st["expt"], axis=mybir.AxisListType.X, op=ALU.add
                    )
                    rden = smalls.tile([P, NJ], F32, tag="rden")
                    nc.vector.reciprocal(rden, den)
                    st["rden"] = rden

                def t4():  # DVE: arden = expt*rden; atp = arden*sinv
                    st = state[b]
                    arden = sm_pool.tile([P, NJ, K], BF16, tag="arden")
                    nc.vector.tensor_tensor(
                        arden,
                        st["expt"],
                        st["rden"].unsqueeze(-1).to_broadcast((P, NJ, K)),
                        ALU.mult,
                    )
                    st["arden"] = arden
                    atp = sm_pool.tile([P, NJ, K], BF16, tag="atp")
                    nc.vector.tensor_tensor(
                        atp,
                        arden,
                        st["sinv"].unsqueeze(-1).to_broadcast((P, NJ, K)),
                        ALU.mult,
                    )
                    st["atp"] = atp

                def t4b():  # DVE: s1[p,k] = sum_j arden over real pixels
                    st = state[b]
                    s1 = smalls.tile([P, K], F32, tag="s1")
                    nc.vector.tensor_reduce(
                        s1,
                        st["arden"][:, 0:9].rearrange("p j k -> p k j"),
                        axis=mybir.AxisListType.X,
                        op=ALU.add,
                    )
                    nc.vector.tensor_tensor(
                        s1[0:48, :], s1[0:48, :], st["arden"][0:48, 9, :],
                        ALU.add,
                    )
                    st["s1"] = s1

                return [t0, t1, t2, t3, t4, t4b]

            def phase2_pieces(b):
                """Epilog of batch b (vlad normalization), as fillers."""
                st = state[b]

                def p0():  # DVE: negd = asum*c - vlad
                    negd = ep_pool.tile([K, D], F32, tag="negd")
                    nc.vector.scalar_tensor_tensor(
                        out=negd,
                        in0=cent_sb,
                        scalar=asum[:, 0:1],
                        in1=vl[:, :],
                        op0=ALU.mult,
                        op1=ALU.subtract,
                    )
                    st["negd"] = negd

                def p1():  # ACT: row sum of squares
                    ssk = ep_pool.tile([K, 1], F32, tag="ssk")
                    nc.scalar.activation(
                        sqj3[:, :], st["negd"], ACTF.Square, accum_out=ssk
                    )
                    st["ssk"] = ssk

                def p2():  # ACT: gk = (1/8)*rsqrt(ssk); Pool: gkn = -gk
                    lssk = ep_pool.tile([K, 1], F32, tag="lssk")
                    nc.scalar.activation(lssk, st["ssk"], ACTF.Ln)
                    gk = ep_pool.tile([K, 1], F32, tag="gk")
                    nc.scalar.activation(
                        gk, lssk, ACTF.Exp, scale=-0.5, bias=ln8
                    )
                    gkn = ep_pool.tile([K, 1], F32, tag="gkn")
                    nc.gpsimd.tensor_scalar(
                        out=gkn, in0=gk, scalar1=-1.0, scalar2=None,
                        op0=ALU.mult,
                    )
                    st["gkn"] = gkn

                def p3():  # ACT: ot = -gk * negd; Pool: output DMA
                    ot = ep_pool.tile([K, D], F32, tag="ot")
                    nc.scalar.activation(
                        ot, st["negd"], ACTF.Copy, scale=st["gkn"]
                    )
                    nc.gpsimd.dma_start(out[b, :, :], ot)
                    state.pop(b)

                return [p0, p1, p2, p3]

            def agg_chunks(b, js):
                st = state[b]
                xt, atp = st["xt"], st["atp"]
                for j in js:
                    nr = NJREAL[j]
                    nc.tensor.matmul(
                        vl,
                        atp[:nr, j],
                        xt[:nr, j, :],
                        start=(j == 0),
                        stop=(j == NJ - 1),
                    )

            def asum_mm(b):
                st = state[b]
                last = nc.tensor.matmul(
                    asum[:, 0:2],
                    st["s1"],
                    onesf[:, 0:2],
                    start=True,
                    stop=True,
                    skip_group_check=True,
                )
                state["last_pe"] = last

            def do_square(b, jq):
                st = state[b]
                nr = NJREAL[jq]
                if SQ_ENG[jq] == "v":
                    nc.vector.scalar_tensor_tensor(
                        out=sqj[:nr],
                        in0=st["xt"][:nr, jq, :],
                        scalar=1.0,
                        in1=st["xt"][:nr, jq, :],
                        op0=ALU.mult,
                        op1=ALU.mult,
                        accum_out=st["ss"][:nr, jq : jq + 1],
                    )
                else:
                    nc.scalar.activation(
                        sqj2[:nr],
                        st["xt"][:nr, jq, :],
                        ACTF.Square,
                        accum_out=st["ss"][:nr, jq : jq + 1],
                    )

            loads = {}

            def emit_loads(b):
                """Queue batch b's x DMAs (natural layout + transposed
                copy).  Called two batches ahead: the sync HWDGE ring moves
                ~9us per batch, so depth-2 prefetch keeps every arrival a
                full batch early."""
                xb = xnat_pool.tile([P, DCH, N], BF16, tag="xb")
                xt = xt_pool.tile([P, NJ, D], BF16, tag="xt")
                nc.sync.dma_start(
                    xb, x[b, :, 0:N].rearrange("(a p) n -> p a n", p=P)
                )
                nc.sync.dma_start(xt, x[b, :, :], transpose=True)
                loads[b] = (xb, xt)

            def phase1(b, fillers):
                xb, xt = loads.pop(b)
                lgsb = sm_pool.tile([K, N], F16, tag="lgsb")
                ss = smalls.tile([P, NJ], F32, tag="ss")
                st = state[b] = {"xt": xt, "ss": ss}

                nc.gpsimd.memset(ss, 1.0)
                if b + 2 < BPC:
                    emit_loads(b + 2)

                def emit_warm(src):
                    warm = nc.tensor.matmul(
                        asum[0:2, 2:4], src, src,
                        start=True, stop=True, skip_group_check=True,
                    )
                    if "last_pe" in state:
                        add_dep_helper(
                            warm.ins, state["last_pe"].ins, sync=False,
                            reason="pin warm after prior PE work",
                        )
                    state["last_pe"] = warm

                def run(seg):
                    for f in fillers.get(seg, ()):
                        f()

                # seg0: this batch's squares run first - xt is prefetched
                # two batches ahead, and finishing ss early lets the whole
                # softmax tail run in-batch, so the aggregation of b-1
                # spreads over segments 2-4 instead of piling into the
                # batch tail.  Batch 0's xt is still in flight, so its
                # squares move after the transposes to keep the ACT/DVE
                # queues from stalling at kernel start.
                # asum matmul of b-2 at the batch head: its s1 input is a
                # full batch old, so it issues without stalling the PE, and
                # it lands just before p0(b-2) reads asum in run(0).
                if b > 1:
                    asum_mm(b - 2)
                emit_warm(xb[:, 0, 0:2])
                for jq in range(0, 5):
                    do_square(b, jq)
                run(0)
                for rg, (c0, c1) in enumerate(NRANGES):
                    for a in range(DCH):
                        last = nc.tensor.matmul(
                            lgT[:, rg, 0 : c1 - c0],
                            wt_sb[:, a, :],
                            xb[:, a, c0:c1],
                            start=(a == 0),
                            stop=(a == DCH - 1),
                            skip_group_check=True,
                        )
                    state["last_pe"] = last
                    if rg == 0:
                        for jq in range(5, NJ):
                            do_square(b, jq)
                    run(rg + 1)
                    if rg == 1:
                        # ranges 0-1 are done: evict their logits (chunks
                        # 0-7) and transpose them n-major while rg2's
                        # matmuls are still queued, so only a 176-col evict
                        # and two transposes remain on the softmax chain
                        # after rg2.  The tiny transpose warm absorbs the
                        # lgnP WAR (vs the previous batch's prescale).
                        nc.scalar.copy(
                            lgsb[:, 0 : 8 * P],
                            lgT.rearrange("k r c -> k (r c)")[:, 0 : 8 * P],
                        )
                        warmt = nc.tensor.matmul(
                            lgnP[0:2, 0, 0:2], identh[:, 0:2],
                            identh[:, 0:2], is_transpose=True,
                            start=True, stop=True, skip_group_check=True,
                        )
                        add_dep_helper(
                            warmt.ins, state["last_pe"].ins, sync=False,
                            reason="pin lgnP warm after prior PE work",
                        )
                        state["last_pe"] = warmt
                        for j in range(0, 8):
                            state["last_pe"] = nc.tensor.matmul(
                                lgnP[:, j, :],
                                lgsb[:, j * P : (j + 1) * P],
                                identh,
                                is_transpose=True, start=True, stop=True,
                                skip_group_check=True,
                            )
                    if b > 0 and rg == 2:
                        agg_chunks(b - 1, range(0, 5))
                # seg4: evict + transpose the last range's logits
                nc.scalar.copy(
                    lgsb[:, 8 * P : N],
                    lgT.rearrange("k r c -> k (r c)")[:, 8 * P : N],
                )
                for j in range(8, NJ):
                    nr = NJREAL[j]
                    last = nc.tensor.matmul(
                        lgnP[0:nr, j, :],
                        lgsb[:, j * P : j * P + nr],
                        identh,
                        is_transpose=True,
                        start=True,
                        stop=True,
                        skip_group_check=True,
                    )
                state["last_pe"] = last
                if b > 0:
                    agg_chunks(b - 1, range(5, NJ))
                run(4)
                run(5)

            emit_loads(0)
            emit_loads(1)
            for b in range(BPC):
                fillers = {}
                t = tail_pieces(b)
                fillers[2] = [t[0]]          # sinv once ss is complete
                fillers[4] = [t[1]]          # prescale after the transposes
                fillers[5] = [t[2], t[3], t[4], t[5]]
                if b > 1:
                    p = phase2_pieces(b - 2)
                    # negd (vl WAR) must precede the first aggregation MM
                    fillers[0] = [p[0]]
                    fillers.setdefault(4, []).insert(0, p[1])
                    fillers[5].extend([p[2], p[3]])
                phase1(b, fillers)
            # drain
            asum_mm(BPC - 2)
            p = phase2_pieces(BPC - 2)
            p[0]()
            agg_chunks(BPC - 1, range(NJ))
            asum_mm(BPC - 1)
            for f in p[1:]:
                f()
            for f in phase2_pieces(BPC - 1):
                f()

    return nc


_NC = None


def _patch_act_tables():
    """Force every ScalarE activation onto the one table set that contains
    {copy, square, ln, exp} so the kernel pays a single ACT_TABLE_LOAD
    instead of thrashing between exp_and_others and natural_log."""
    import concourse.bacc as _bacc_mod
    orig = _bacc_mod.get_activation_tables

    def patched(arch):
        tables = dict(orig(arch))
        assert "natural_log_exp_and_others" in tables
        return {
            name: (funcs if name == "natural_log_exp_and_others" else set())
            for name, funcs in tables.items()
        }

    _bacc_mod.get_activation_tables = patched


def _get_nc():
    global _NC
    if _NC is None:
        _patch_act_tables()
        nc = bacc.Bacc("TRN2", target_bir_lowering=False)
        _emit(nc)
        nc.compile()
        _NC = nc
    return _NC


def _make_in_maps(x, conv_w, centroids):
    import ml_dtypes

    bf16 = ml_dtypes.bfloat16
    B = x.shape[0]
    xp = np.zeros((B, D, NP), dtype=bf16)
    xp[:, :, 0:N] = np.asarray(x, dtype=np.float32).reshape(B, D, N).astype(bf16)
    wt = np.ascontiguousarray(np.asarray(conv_w.T, dtype=np.float32).astype(bf16))
    cent = np.ascontiguousarray(centroids, dtype=np.float32)
    in_maps = []
    for c in range(8):
        in_maps.append(
            {
                "x": np.ascontiguousarray(xp[c * BPC : (c + 1) * BPC]),
                "wt": wt,
                "cent": cent,
            }
        )
    return in_maps


def _run(x, conv_w, centroids, trace=False):
    nc = _get_nc()
    res = run_bass_kernel_spmd(
        nc,
        _make_in_maps(x, conv_w, centroids),
        core_ids=list(range(8)),
        trace=trace,
    )
    outs = [r["out"].reshape(BPC, K * D) for r in res.results]
    full = np.concatenate(outs, axis=0)
    return full, res


def kernel(x, conv_w, centroids):
    full, _ = _run(x, conv_w, centroids, trace=False)
    return full


# revision 79
# speedup vs baseline: 1.0702x; 1.0702x over previous
"""NetVLAD Trainium2 kernel (Bass/Tile), data-parallel over batch on 8 cores.

Math (per batch b):
    x_hat = x / ||x||_2(channel)                    (B, D, H*W), D=512, N=1200
    logits = conv_w @ x_hat                         (K, N), K=64
    a = softmax_K(logits)
    vlad[k,d] = sum_n a[k,n] * x_hat[d,n] - (sum_n a[k,n]) * c[k,d]
    vlad = l2norm_rows(vlad); out = l2norm(flatten(vlad))   # == vlad_rows/8

Device-side structure (v8, DMA-transpose):
  - x is staged host-side as bf16 padded to N=1280 and DMA'd twice per
    batch: once in natural d-major layout (3 n-range parts) for the logits
    matmuls, and once through the DMA TRANSPOSE XBAR (16x128 tiles) into
    xt[p, j, d] = x[d, 10p+j].  This removes every PE transpose and every
    per-chunk PSUM eviction of the old design.  Pad pixels (n >= 1200)
    live in partitions 120:128 of every chunk and are zero.
  - logits are computed k-major: lgT[64, n] = sum_d wt[d,k] x[d,n], with
    wt chunks stationary and 512-wide bf16 moving x slices (1 cyc/row),
    accumulating over the 4 d-chunks into PSUM [64, 1200].  One ACT copy
    evicts lgT to fp16, and a second (SBUF->SBUF) DMA transpose turns it
    into n-major lgn[p, j, k] with the same 10p+j pixel mapping, ready for
    the batched n-major softmax tail.
  - softmax tail unchanged in spirit: sinv = exp(-0.5 ln(ss)); lgsc =
    lgn * sinv; exp; den; arden = expt*rden; atp = arden*sinv (bf16).
  - aggregation: vl[k,d] += atp_j^T @ xt_j over 10 chunks (bf16, 512-wide
    moving).  asum comes from s1[p,k] = sum_j arden (DVE reduce over the
    real partitions) + a tiny ones-moving matmul reducing partitions.
  - ss: 10 Square/STT accum passes over xt[0:120] (the real pixels),
    split ACT/DVE; ss is memset to 1.0 so pad lanes stay finite.
  - PSUM: lgT [64, 2, 1536] (2 parities x 3 bank-aligned 512-col matmul
    dests) + vl + asum = 8 banks.  The only PSUM recycling is the lgT
    parity, reused every other batch - no per-chunk rotation, no
    starvation coupling.
  - Warm matmuls (dest: junk cols of the asum bank) absorb the x DMA part
    semaphores so each first range matmul carries only the lgT parity WAR
    (walrus S3_LW allows one sync wait per Matmult).
  - rsqrt as exp(-0.5*ln), single ACT table set, gpsimd for tiny ops and
    the output DMA, software pipeline: tail of b-1 and epilog of b-2 run
    interleaved with batch b's matmuls.
"""

import numpy as np

import concourse.bass as bass
import concourse.mybir as mybir
from concourse import bacc
import concourse.tile as tile
from concourse.bass_utils import run_bass_kernel_spmd
from concourse.masks import make_identity
from concourse.tile_rust import add_dep_helper

F32 = mybir.dt.float32
F16 = mybir.dt.float16
BF16 = mybir.dt.bfloat16
ALU = mybir.AluOpType
ACTF = mybir.ActivationFunctionType

P = 128
BPC = 8            # batches per core
D = 512
N = 1200
NP = 1280          # padded pixel count (XBAR needs free % 128 == 0)
K = 64
DCH = D // P       # 4 d-chunks
NJ = 10            # pixel chunks; xt[p, j, :] = x[:, 128j + p]
NJREAL = [P] * 9 + [48]   # real partitions per chunk (n < 1200)
NRANGES = [(0, 512), (512, 1024), (1024, 1200)]
LN_EIGHTH = float(np.log(0.125))

SQ_ENG = "v a v v a v a v a v".split()


def _emit(nc):
    x = nc.dram_tensor("x", (BPC, D, NP), BF16, kind="ExternalInput")
    wt = nc.dram_tensor("wt", (D, K), BF16, kind="ExternalInput")
    cent = nc.dram_tensor("cent", (K, D), F32, kind="ExternalInput")
    out = nc.dram_tensor("out", (BPC, K, D), F32, kind="ExternalOutput")

    with tile.TileContext(nc) as tc:
        with (
            tc.tile_pool(name="const", bufs=1) as const,
            tc.tile_pool(name="xnat", bufs=4) as xnat_pool,
            tc.tile_pool(name="xtsb", bufs=4) as xt_pool,
            tc.tile_pool(name="softmax", bufs=2) as sm_pool,
            tc.tile_pool(name="smalls", bufs=2) as smalls,
            tc.tile_pool(name="epilog", bufs=2) as ep_pool,
            tc.tile_pool(name="psum", bufs=1, space="PSUM") as psum,
        ):
            wt_sb = const.tile([P, DCH, K], BF16)
            nc.sync.dma_start(wt_sb, wt[:, :].rearrange("(a p) k -> p a k", p=P))
            cent_sb = const.tile([K, D], F32)
            nc.sync.dma_start(cent_sb, cent[:, :])
            ln8 = const.tile([K, 1], F32)
            nc.gpsimd.memset(ln8, LN_EIGHTH)
            onesf = const.tile([P, 2], F32)
            nc.gpsimd.memset(onesf, 1.0)
            identf = const.tile([K, K], F32)
            make_identity(nc, identf)
            identh = const.tile([K, K], F16)
            nc.vector.tensor_copy(identh, identf)
            # never-read junk outputs for square-accumulate passes
            sqj = const.tile([P, D], BF16)
            sqj2 = const.tile([P, D], BF16)
            sqj3 = const.tile([K, D], BF16)

            # PSUM (6 of 8 banks): k-major logits (3 bank-aligned 512-col
            # matmul dests, no parity - the eviction happens well before the
            # next batch's first logits matmul), the n-major transposed
            # logits (fp16, written by PE transposes, read by the DVE
            # prescale), vlad, asum (+junk cols for warms).
            lgT = psum.tile([K, 3, 512], F32)      # 3 banks
            lgnP = psum.tile([P, NJ, K], F16)      # 1 bank
            vl = psum.tile([K, D], F32)            # 1 bank
            asum = psum.tile([K, 4], F32)          # 1 bank
            nc.vector.memset(lgnP.bitcast(F32), 0.0)

            # Startup warms: absorb the wt DMA / onesf memset / identh copy
            # semaphores so no first real matmul carries more than one wait.
            w0 = nc.tensor.matmul(
                asum[0:2, 2:4], wt_sb[:, 3, 0:2], wt_sb[:, 3, 0:2],
                start=True, stop=True, skip_group_check=True,
            )
            w1 = nc.tensor.matmul(
                asum[0:2, 2:4], onesf[:, 0:2], onesf[:, 0:2],
                start=True, stop=True, skip_group_check=True,
            )
            add_dep_helper(w1.ins, w0.ins, sync=False, reason="warm chain")
            w2 = nc.tensor.matmul(
                lgnP[0:2, 0, 0:2], identh[:, 0:2], identh[:, 0:2],
                is_transpose=True, start=True, stop=True,
                skip_group_check=True,
            )
            add_dep_helper(w2.ins, w1.ins, sync=False, reason="warm chain")

            state = {}

            def tail_pieces(b):
                """Softmax tail of batch b (runs in-batch as fillers)."""

                def t0():  # ACT: sinv = exp(-0.5*ln(ss)) (fp16 for DVE 2x)
                    st = state[b]
                    lss = smalls.tile([P, NJ], F32, tag="lss")
                    nc.scalar.activation(lss, st["ss"], ACTF.Ln)
                    sinv = smalls.tile([P, NJ], F16, tag="sinv")
                    nc.scalar.activation(sinv, lss, ACTF.Exp, scale=-0.5)
                    st["sinv"] = sinv

                def t1():  # DVE: prescale logits (reads the fp16 PSUM bank)
                    st = state[b]
                    lgsc = sm_pool.tile([P, NJ, K], BF16, tag="lgsc")
                    nc.vector.tensor_tensor(
                        lgsc,
                        lgnP[:, :, :],
                        st["sinv"].unsqueeze(-1).to_broadcast((P, NJ, K)),
                        ALU.mult,
                    )
                    st["lgsc"] = lgsc

                def t2():  # ACT: one big exp
                    st = state[b]
                    expt = sm_pool.tile([P, NJ, K], BF16, tag="expt")
                    nc.scalar.activation(expt, st["lgsc"], ACTF.Exp)
                    st["expt"] = expt

                def t3():  # DVE: denominators
                    st = state[b]
                    den = smalls.tile([P, NJ], F32, tag="den")
                    nc.vector.tensor_reduce(
                        den, st["expt"], axis=mybir.AxisListType.X, op=ALU.add
                    )
                    rden = smalls.tile([P, NJ], F32, tag="rden")
                    nc.vector.reciprocal(rden, den)
                    st["rden"] = rden

                def t4():  # DVE: arden = expt*rden; atp = arden*sinv
                    st = state[b]
                    arden = sm_pool.tile([P, NJ, K], BF16, tag="arden")
                    nc.vector.tensor_tensor(
                        arden,
                        st["expt"],
                        st["rden"].unsqueeze(-1).to_broadcast((P, NJ, K)),
                        ALU.mult,
                    )
                    st["arden"] = arden
                    atp = sm_pool.tile([P, NJ, K], BF16, tag="atp")
                    nc.vector.tensor_tensor(
                        atp,
                        arden,
                        st["sinv"].unsqueeze(-1).to_broadcast((P, NJ, K)),
                        ALU.mult,
                    )
                    st["atp"] = atp

                def t4b():  # DVE: s1[p,k] = sum_j arden over real pixels
                    st = state[b]
                    s1 = smalls.tile([P, K], F32, tag="s1")
                    nc.vector.tensor_reduce(
                        s1,
                        st["arden"][:, 0:9].rearrange("p j k -> p k j"),
                        axis=mybir.AxisListType.X,
                        op=ALU.add,
                    )
                    nc.vector.tensor_tensor(
                        s1[0:48, :], s1[0:48, :], st["arden"][0:48, 9, :],
                        ALU.add,
                    )
                    st["s1"] = s1

                return [t0, t1, t2, t3, t4, t4b]

            def phase2_pieces(b):
                """Epilog of batch b (vlad normalization), as fillers."""
                st = state[b]

                def p0():  # DVE: negd = asum*c - vlad
                    negd = ep_pool.tile([K, D], F32, tag="negd")
                    nc.vector.scalar_tensor_tensor(
                        out=negd,
                        in0=cent_sb,
                        scalar=asum[:, 0:1],
                        in1=vl[:, :],
                        op0=ALU.mult,
                        op1=ALU.subtract,
                    )
                    st["negd"] = negd

                def p1():  # ACT: row sum of squares
                    ssk = ep_pool.tile([K, 1], F32, tag="ssk")
                    nc.scalar.activation(
                        sqj3[:, :], st["negd"], ACTF.Square, accum_out=ssk
                    )
                    st["ssk"] = ssk

                def p2():  # ACT: gk = (1/8)*rsqrt(ssk); Pool: gkn = -gk
                    lssk = ep_pool.tile([K, 1], F32, tag="lssk")
                    nc.scalar.activation(lssk, st["ssk"], ACTF.Ln)
                    gk = ep_pool.tile([K, 1], F32, tag="gk")
                    nc.scalar.activation(
                        gk, lssk, ACTF.Exp, scale=-0.5, bias=ln8
                    )
                    gkn = ep_pool.tile([K, 1], F32, tag="gkn")
                    nc.gpsimd.tensor_scalar(
                        out=gkn, in0=gk, scalar1=-1.0, scalar2=None,
                        op0=ALU.mult,
                    )
                    st["gkn"] = gkn

                def p3():  # ACT: ot = -gk * negd; Pool: output DMA
                    ot = ep_pool.tile([K, D], F32, tag="ot")
                    nc.scalar.activation(
                        ot, st["negd"], ACTF.Copy, scale=st["gkn"]
                    )
                    nc.gpsimd.dma_start(out[b, :, :], ot)
                    state.pop(b)

                return [p0, p1, p2, p3]

            def agg_chunks(b, js):
                st = state[b]
                xt, atp = st["xt"], st["atp"]
                for j in js:
                    nr = NJREAL[j]
                    nc.tensor.matmul(
                        vl,
                        atp[:nr, j],
                        xt[:nr, j, :],
                        start=(j == 0),
                        stop=(j == NJ - 1),
                    )

            def asum_mm(b):
                st = state[b]
                last = nc.tensor.matmul(
                    asum[:, 0:2],
                    st["s1"],
                    onesf[:, 0:2],
                    start=True,
                    stop=True,
                    skip_group_check=True,
                )
                state["last_pe"] = last

            def do_square(b, jq):
                st = state[b]
                nr = NJREAL[jq]
                if SQ_ENG[jq] == "v":
                    nc.vector.scalar_tensor_tensor(
                        out=sqj[:nr],
                        in0=st["xt"][:nr, jq, :],
                        scalar=1.0,
                        in1=st["xt"][:nr, jq, :],
                        op0=ALU.mult,
                        op1=ALU.mult,
                        accum_out=st["ss"][:nr, jq : jq + 1],
                    )
                else:
                    nc.scalar.activation(
                        sqj2[:nr],
                        st["xt"][:nr, jq, :],
                        ACTF.Square,
                        accum_out=st["ss"][:nr, jq : jq + 1],
                    )

            loads = {}

            def emit_loads(b):
                """Queue batch b's x DMAs (natural layout + transposed
                copy).  Called two batches ahead: the sync HWDGE ring moves
                ~9us per batch, so depth-2 prefetch keeps every arrival a
                full batch early."""
                xb = xnat_pool.tile([P, DCH, N], BF16, tag="xb")
                xt = xt_pool.tile([P, NJ, D], BF16, tag="xt")
                nc.sync.dma_start(
                    xb, x[b, :, 0:N].rearrange("(a p) n -> p a n", p=P)
                )
                nc.sync.dma_start(xt, x[b, :, :], transpose=True)
                loads[b] = (xb, xt)

            def phase1(b, fillers):
                xb, xt = loads.pop(b)
                lgsb = sm_pool.tile([K, N], F16, tag="lgsb")
                ss = smalls.tile([P, NJ], F32, tag="ss")
                st = state[b] = {"xt": xt, "ss": ss}

                nc.gpsimd.memset(ss, 1.0)
                if b + 2 < BPC:
                    emit_loads(b + 2)

                def emit_warm(src):
                    warm = nc.tensor.matmul(
                        asum[0:2, 2:4], src, src,
                        start=True, stop=True, skip_group_check=True,
                    )
                    if "last_pe" in state:
                        add_dep_helper(
                            warm.ins, state["last_pe"].ins, sync=False,
                            reason="pin warm after prior PE work",
                        )
                    state["last_pe"] = warm

                def run(seg):
                    for f in fillers.get(seg, ()):
                        f()

                # seg0: this batch's squares run first - xt is prefetched
                # two batches ahead, and finishing ss early lets the whole
                # softmax tail run in-batch, so the aggregation of b-1
                # spreads over segments 2-4 instead of piling into the
                # batch tail.  Batch 0's xt is still in flight, so its
                # squares move after the transposes to keep the ACT/DVE
                # queues from stalling at kernel start.
                # asum matmul of b-2 at the batch head: its s1 input is a
                # full batch old, so it issues without stalling the PE, and
                # it lands just before p0(b-2) reads asum in run(0).
                if b > 1:
                    asum_mm(b - 2)
                emit_warm(xb[:, 0, 0:2])
                for jq in range(0, 5):
                    do_square(b, jq)
                run(0)
                for rg, (c0, c1) in enumerate(NRANGES):
                    for a in range(DCH):
                        last = nc.tensor.matmul(
                            lgT[:, rg, 0 : c1 - c0],
                            wt_sb[:, a, :],
                            xb[:, a, c0:c1],
                            start=(a == 0),
                            stop=(a == DCH - 1),
                            skip_group_check=True,
                        )
                    state["last_pe"] = last
                    if rg == 0:
                        for jq in range(5, NJ):
                            do_square(b, jq)
                    run(rg + 1)
                    if b > 0 and rg == 2:
                        agg_chunks(b - 1, range(0, 5))
                # seg4: evict logits to fp16 on ACT; a tiny transpose warm
                # absorbs the lgnP WAR (vs the previous batch's prescale) so
                # each real transpose carries only the eviction semaphore.
                nc.scalar.copy(
                    lgsb,
                    lgT.rearrange("k r c -> k (r c)")[:, 0:N],
                )
                warmt = nc.tensor.matmul(
                    lgnP[0:2, 0, 0:2], identh[:, 0:2], identh[:, 0:2],
                    is_transpose=True, start=True, stop=True,
                    skip_group_check=True,
                )
                add_dep_helper(
                    warmt.ins, state["last_pe"].ins, sync=False,
                    reason="pin lgnP warm after prior PE work",
                )
                state["last_pe"] = warmt
                for j in range(NJ):
                    nr = NJREAL[j]
                    last = nc.tensor.matmul(
                        lgnP[0:nr, j, :],
                        lgsb[:, j * P : j * P + nr],
                        identh,
                        is_transpose=True,
                        start=True,
                        stop=True,
                        skip_group_check=True,
                    )
                state["last_pe"] = last
                if b > 0:
                    agg_chunks(b - 1, range(5, NJ))
                run(4)
                run(5)

            emit_loads(0)
            emit_loads(1)
            for b in range(BPC):
                fillers = {}
                t = tail_pieces(b)
                fillers[2] = [t[0]]          # sinv once ss is complete
                fillers[4] = [t[1]]          # prescale after the transposes
                fillers[5] = [t[2], t[3], t[4], t[5]]
                if b > 1:
                    p = phase2_pieces(b - 2)
                    # negd (vl WAR) must precede the first aggregation MM
                    fillers[0] = [p[0]]
                    fillers.setdefault(4, []).insert(0, p[1])
                    fillers[5].extend([p[2], p[3]])
                phase1(b, fillers)
            # drain
            asum_mm(BPC - 2)
            p = phase2_pieces(BPC - 2)
            p[0]()
            agg_chunks(BPC - 1, range(NJ))
            asum_mm(BPC - 1)
            for f in p[1:]:
                f()
            for f in phase2_pieces(BPC - 1):
                f()

    return nc


_NC = None


def _patch_act_tables():
    """Force every ScalarE activation onto the one table set that contains
    {copy, square, ln, exp} so the kernel pays a single ACT_TABLE_LOAD
    instead of thrashing between exp_and_others and natural_log."""
    import concourse.bacc as _bacc_mod
    orig = _bacc_mod.get_activation_tables

    def patched(arch):
        tables = dict(orig(arch))
        assert "natural_log_exp_and_others" in tables
        return {
            name: (funcs if name == "natural_log_exp_and_others" else set())
            for name, funcs in tables.items()
        }

    _bacc_mod.get_activation_tables = patched


def _get_nc():
    global _NC
    if _NC is None:
        _patch_act_tables()
        nc = bacc.Bacc("TRN2", target_bir_lowering=False)
        _emit(nc)
        nc.compile()
        _NC = nc
    return _NC


def _make_in_maps(x, conv_w, centroids):
    import ml_dtypes

    bf16 = ml_dtypes.bfloat16
    B = x.shape[0]
    xp = np.zeros((B, D, NP), dtype=bf16)
    xp[:, :, 0:N] = np.asarray(x, dtype=np.float32).reshape(B, D, N).astype(bf16)
    wt = np.ascontiguousarray(np.asarray(conv_w.T, dtype=np.float32).astype(bf16))
    cent = np.ascontiguousarray(centroids, dtype=np.float32)
    in_maps = []
    for c in range(8):
        in_maps.append(
            {
                "x": np.ascontiguousarray(xp[c * BPC : (c + 1) * BPC]),
                "wt": wt,
                "cent": cent,
            }
        )
    return in_maps


def _run(x, conv_w, centroids, trace=False):
    nc = _get_nc()
    res = run_bass_kernel_spmd(
        nc,
        _make_in_maps(x, conv_w, centroids),
        core_ids=list(range(8)),
        trace=trace,
    )
    outs = [r["out"].reshape(BPC, K * D) for r in res.results]
    full = np.concatenate(outs, axis=0)
    return full, res


def kernel(x, conv_w, centroids):
    full, _ = _run(x, conv_w, centroids, trace=False)
    return full


# revision 83
# speedup vs baseline: 1.0706x; 1.0004x over previous
"""NetVLAD Trainium2 kernel (Bass/Tile), data-parallel over batch on 8 cores.

Math (per batch b):
    x_hat = x / ||x||_2(channel)                    (B, D, H*W), D=512, N=1200
    logits = conv_w @ x_hat                         (K, N), K=64
    a = softmax_K(logits)
    vlad[k,d] = sum_n a[k,n] * x_hat[d,n] - (sum_n a[k,n]) * c[k,d]
    vlad = l2norm_rows(vlad); out = l2norm(flatten(vlad))   # == vlad_rows/8

Device-side structure (v8, DMA-transpose):
  - x is staged host-side as bf16 padded to N=1280 and DMA'd twice per
    batch: once in natural d-major layout (3 n-range parts) for the logits
    matmuls, and once through the DMA TRANSPOSE XBAR (16x128 tiles) into
    xt[p, j, d] = x[d, 10p+j].  This removes every PE transpose and every
    per-chunk PSUM eviction of the old design.  Pad pixels (n >= 1200)
    live in partitions 120:128 of every chunk and are zero.
  - logits are computed k-major: lgT[64, n] = sum_d wt[d,k] x[d,n], with
    wt chunks stationary and 512-wide bf16 moving x slices (1 cyc/row),
    accumulating over the 4 d-chunks into PSUM [64, 1200].  One ACT copy
    evicts lgT to fp16, and a second (SBUF->SBUF) DMA transpose turns it
    into n-major lgn[p, j, k] with the same 10p+j pixel mapping, ready for
    the batched n-major softmax tail.
  - softmax tail unchanged in spirit: sinv = exp(-0.5 ln(ss)); lgsc =
    lgn * sinv; exp; den; arden = expt*rden; atp = arden*sinv (bf16).
  - aggregation: vl[k,d] += atp_j^T @ xt_j over 10 chunks (bf16, 512-wide
    moving).  asum comes from s1[p,k] = sum_j arden (DVE reduce over the
    real partitions) + a tiny ones-moving matmul reducing partitions.
  - ss: 10 Square/STT accum passes over xt[0:120] (the real pixels),
    split ACT/DVE; ss is memset to 1.0 so pad lanes stay finite.
  - PSUM: lgT [64, 2, 1536] (2 parities x 3 bank-aligned 512-col matmul
    dests) + vl + asum = 8 banks.  The only PSUM recycling is the lgT
    parity, reused every other batch - no per-chunk rotation, no
    starvation coupling.
  - Warm matmuls (dest: junk cols of the asum bank) absorb the x DMA part
    semaphores so each first range matmul carries only the lgT parity WAR
    (walrus S3_LW allows one sync wait per Matmult).
  - rsqrt as exp(-0.5*ln), single ACT table set, gpsimd for tiny ops and
    the output DMA, software pipeline: tail of b-1 and epilog of b-2 run
    interleaved with batch b's matmuls.
"""

import numpy as np

import concourse.bass as bass
import concourse.mybir as mybir
from concourse import bacc
import concourse.tile as tile
from concourse.bass_utils import run_bass_kernel_spmd
from concourse.masks import make_identity
from concourse.tile_rust import add_dep_helper

F32 = mybir.dt.float32
F16 = mybir.dt.float16
BF16 = mybir.dt.bfloat16
ALU = mybir.AluOpType
ACTF = mybir.ActivationFunctionType

P = 128
BPC = 8            # batches per core
D = 512
N = 1200
NP = 1280          # padded pixel count (XBAR needs free % 128 == 0)
K = 64
DCH = D // P       # 4 d-chunks
NJ = 10            # pixel chunks; xt[p, j, :] = x[:, 128j + p]
NJREAL = [P] * 9 + [48]   # real partitions per chunk (n < 1200)
NRANGES = [(0, 512), (512, 1024), (1024, 1200)]
LN_EIGHTH = float(np.log(0.125))

SQ_ENG = "v a v v a v a v a v".split()


def _emit(nc):
    x = nc.dram_tensor("x", (BPC, D, NP), BF16, kind="ExternalInput")
    wt = nc.dram_tensor("wt", (D, K), BF16, kind="ExternalInput")
    cent = nc.dram_tensor("cent", (K, D), F32, kind="ExternalInput")
    out = nc.dram_tensor("out", (BPC, K, D), F32, kind="ExternalOutput")

    with tile.TileContext(nc) as tc:
        with (
            tc.tile_pool(name="const", bufs=1) as const,
            tc.tile_pool(name="xnat", bufs=4) as xnat_pool,
            tc.tile_pool(name="xtsb", bufs=4) as xt_pool,
            tc.tile_pool(name="softmax", bufs=2) as sm_pool,
            tc.tile_pool(name="smalls", bufs=2) as smalls,
            tc.tile_pool(name="epilog", bufs=2) as ep_pool,
            tc.tile_pool(name="psum", bufs=1, space="PSUM") as psum,
        ):
            wt_sb = const.tile([P, DCH, K], BF16)
            nc.sync.dma_start(wt_sb, wt[:, :].rearrange("(a p) k -> p a k", p=P))
            cent_sb = const.tile([K, D], F32)
            nc.sync.dma_start(cent_sb, cent[:, :])
            ln8 = const.tile([K, 1], F32)
            nc.gpsimd.memset(ln8, LN_EIGHTH)
            onesf = const.tile([P, 2], F32)
            nc.gpsimd.memset(onesf, 1.0)
            identf = const.tile([K, K], F32)
            make_identity(nc, identf)
            identh = const.tile([K, K], F16)
            nc.vector.tensor_copy(identh, identf)
            # never-read junk outputs for square-accumulate passes
            sqj = const.tile([P, D], BF16)
            sqj2 = const.tile([P, D], BF16)
            sqj3 = const.tile([K, D], BF16)

            # PSUM (6 of 8 banks): k-major logits (3 bank-aligned 512-col
            # matmul dests, no parity - the eviction happens well before the
            # next batch's first logits matmul), the n-major transposed
            # logits (fp16, written by PE transposes, read by the DVE
            # prescale), vlad, asum (+junk cols for warms).
            lgT = psum.tile([K, 3, 512], F32)      # 3 banks
            lgnP = psum.tile([P, NJ, K], F16)      # 1 bank
            vl = psum.tile([K, D], F32)            # 1 bank
            asum = psum.tile([K, 4], F32)          # 1 bank
            nc.vector.memset(lgnP.bitcast(F32), 0.0)

            # Startup warms: absorb the wt DMA / onesf memset / identh copy
            # semaphores so no first real matmul carries more than one wait.
            w0 = nc.tensor.matmul(
                asum[0:2, 2:4], wt_sb[:, 3, 0:2], wt_sb[:, 3, 0:2],
                start=True, stop=True, skip_group_check=True,
            )
            w1 = nc.tensor.matmul(
                asum[0:2, 2:4], onesf[:, 0:2], onesf[:, 0:2],
                start=True, stop=True, skip_group_check=True,
            )
            add_dep_helper(w1.ins, w0.ins, sync=False, reason="warm chain")
            w2 = nc.tensor.matmul(
                lgnP[0:2, 0, 0:2], identh[:, 0:2], identh[:, 0:2],
                is_transpose=True, start=True, stop=True,
                skip_group_check=True,
            )
            add_dep_helper(w2.ins, w1.ins, sync=False, reason="warm chain")

            state = {}

            def tail_pieces(b):
                """Softmax tail of batch b (runs in-batch as fillers)."""

                def t0():  # ACT: sinv = exp(-0.5*ln(ss)) (fp16 for DVE 2x)
                    st = state[b]
                    lss = smalls.tile([P, NJ], F32, tag="lss")
                    nc.scalar.activation(lss, st["ss"], ACTF.Ln)
                    sinv = smalls.tile([P, NJ], F16, tag="sinv")
                    nc.scalar.activation(sinv, lss, ACTF.Exp, scale=-0.5)
                    st["sinv"] = sinv

                def t1():  # DVE: prescale logits (reads the fp16 PSUM bank)
                    st = state[b]
                    lgsc = sm_pool.tile([P, NJ, K], BF16, tag="lgsc")
                    nc.vector.tensor_tensor(
                        lgsc,
                        lgnP[:, :, :],
                        st["sinv"].unsqueeze(-1).to_broadcast((P, NJ, K)),
                        ALU.mult,
                    )
                    st["lgsc"] = lgsc

                def t2():  # ACT: one big exp
                    st = state[b]
                    expt = sm_pool.tile([P, NJ, K], BF16, tag="expt")
                    nc.scalar.activation(expt, st["lgsc"], ACTF.Exp)
                    st["expt"] = expt

                def t3():  # DVE: denominators
                    st = state[b]
                    den = smalls.tile([P, NJ], F32, tag="den")
                    nc.vector.tensor_reduce(
                        den, st["expt"], axis=mybir.AxisListType.X, op=ALU.add
                    )
                    rden = smalls.tile([P, NJ], F32, tag="rden")
                    nc.vector.reciprocal(rden, den)
                    st["rden"] = rden

                def t4():  # DVE: arden = expt*rden; atp = arden*sinv
                    st = state[b]
                    arden = sm_pool.tile([P, NJ, K], BF16, tag="arden")
                    nc.vector.tensor_tensor(
                        arden,
                        st["expt"],
                        st["rden"].unsqueeze(-1).to_broadcast((P, NJ, K)),
                        ALU.mult,
                    )
                    st["arden"] = arden
                    atp = sm_pool.tile([P, NJ, K], BF16, tag="atp")
                    nc.vector.tensor_tensor(
                        atp,
                        arden,
                        st["sinv"].unsqueeze(-1).to_broadcast((P, NJ, K)),
                        ALU.mult,
                    )
                    st["atp"] = atp

                def t4b():  # DVE: s1[p,k] = sum_j arden over real pixels
                    st = state[b]
                    s1 = smalls.tile([P, K], F32, tag="s1")
                    nc.vector.tensor_reduce(
                        s1,
                        st["arden"][:, 0:9].rearrange("p j k -> p k j"),
                        axis=mybir.AxisListType.X,
                        op=ALU.add,
                    )
                    nc.vector.tensor_tensor(
                        s1[0:48, :], s1[0:48, :], st["arden"][0:48, 9, :],
                        ALU.add,
                    )
                    st["s1"] = s1

                return [t0, t1, t2, t3, t4, t4b]

            def phase2_pieces(b):
                """Epilog of batch b (vlad normalization), as fillers."""
                st = state[b]

                def p0():  # DVE: negd = asum*c - vlad
                    negd = ep_pool.tile([K, D], F32, tag="negd")
                    nc.vector.scalar_tensor_tensor(
                        out=negd,
                        in0=cent_sb,
                        scalar=asum[:, 0:1],
                        in1=vl[:, :],
                        op0=ALU.mult,
                        op1=ALU.subtract,
                    )
                    st["negd"] = negd

                def p1():  # ACT: row sum of squares
                    ssk = ep_pool.tile([K, 1], F32, tag="ssk")
                    nc.scalar.activation(
                        sqj3[:, :], st["negd"], ACTF.Square, accum_out=ssk
                    )
                    st["ssk"] = ssk

                def p2():  # ACT: gk = (1/8)*rsqrt(ssk); Pool: gkn = -gk
                    lssk = ep_pool.tile([K, 1], F32, tag="lssk")
                    nc.scalar.activation(lssk, st["ssk"], ACTF.Ln)
                    gk = ep_pool.tile([K, 1], F32, tag="gk")
                    nc.scalar.activation(
                        gk, lssk, ACTF.Exp, scale=-0.5, bias=ln8
                    )
                    gkn = ep_pool.tile([K, 1], F32, tag="gkn")
                    nc.gpsimd.tensor_scalar(
                        out=gkn, in0=gk, scalar1=-1.0, scalar2=None,
                        op0=ALU.mult,
                    )
                    st["gkn"] = gkn

                def p3():  # ACT: ot = -gk * negd; Pool: output DMA
                    ot = ep_pool.tile([K, D], F32, tag="ot")
                    nc.scalar.activation(
                        ot, st["negd"], ACTF.Copy, scale=st["gkn"]
                    )
                    nc.gpsimd.dma_start(out[b, :, :], ot)
                    state.pop(b)

                return [p0, p1, p2, p3]

            def agg_chunks(b, js):
                st = state[b]
                xt, atp = st["xt"], st["atp"]
                for j in js:
                    nr = NJREAL[j]
                    nc.tensor.matmul(
                        vl,
                        atp[:nr, j],
                        xt[:nr, j, :],
                        start=(j == 0),
                        stop=(j == NJ - 1),
                    )

            def asum_mm(b):
                st = state[b]
                last = nc.tensor.matmul(
                    asum[:, 0:2],
                    st["s1"],
                    onesf[:, 0:2],
                    start=True,
                    stop=True,
                    skip_group_check=True,
                )
                state["last_pe"] = last

            def do_square(b, jq):
                st = state[b]
                nr = NJREAL[jq]
                if SQ_ENG[jq] == "v":
                    nc.vector.scalar_tensor_tensor(
                        out=sqj[:nr],
                        in0=st["xt"][:nr, jq, :],
                        scalar=1.0,
                        in1=st["xt"][:nr, jq, :],
                        op0=ALU.mult,
                        op1=ALU.mult,
                        accum_out=st["ss"][:nr, jq : jq + 1],
                    )
                else:
                    nc.scalar.activation(
                        sqj2[:nr],
                        st["xt"][:nr, jq, :],
                        ACTF.Square,
                        accum_out=st["ss"][:nr, jq : jq + 1],
                    )

            loads = {}

            def emit_loads(b):
                """Queue batch b's x DMAs (natural layout + transposed
                copy).  Called two batches ahead: the sync HWDGE ring moves
                ~9us per batch, so depth-2 prefetch keeps every arrival a
                full batch early."""
                xb = xnat_pool.tile([P, DCH, N], BF16, tag="xb")
                xt = xt_pool.tile([P, NJ, D], BF16, tag="xt")
                nc.sync.dma_start(
                    xb, x[b, :, 0:N].rearrange("(a p) n -> p a n", p=P)
                )
                nc.sync.dma_start(xt, x[b, :, :], transpose=True)
                loads[b] = (xb, xt)

            def phase1(b, fillers):
                xb, xt = loads.pop(b)
                lgsb = sm_pool.tile([K, N], F16, tag="lgsb")
                ss = smalls.tile([P, NJ], F32, tag="ss")
                st = state[b] = {"xt": xt, "ss": ss}

                nc.gpsimd.memset(ss, 1.0)
                if b + 2 < BPC:
                    emit_loads(b + 2)

                def emit_warm(src):
                    warm = nc.tensor.matmul(
                        asum[0:2, 2:4], src, src,
                        start=True, stop=True, skip_group_check=True,
                    )
                    if "last_pe" in state:
                        add_dep_helper(
                            warm.ins, state["last_pe"].ins, sync=False,
                            reason="pin warm after prior PE work",
                        )
                    state["last_pe"] = warm

                def run(seg):
                    for f in fillers.get(seg, ()):
                        f()

                # seg0: this batch's squares run first - xt is prefetched
                # two batches ahead, and finishing ss early lets the whole
                # softmax tail run in-batch, so the aggregation of b-1
                # spreads over segments 2-4 instead of piling into the
                # batch tail.  Batch 0's xt is still in flight, so its
                # squares move after the transposes to keep the ACT/DVE
                # queues from stalling at kernel start.
                # asum matmul of b-2 at the batch head: its s1 input is a
                # full batch old, so it issues without stalling the PE, and
                # it lands just before p0(b-2) reads asum in run(0).
                if b > 1:
                    asum_mm(b - 2)
                emit_warm(xb[:, 0, 0:2])
                if b > 0:
                    for jq in range(0, 5):
                        do_square(b, jq)
                run(0)
                for rg, (c0, c1) in enumerate(NRANGES):
                    for a in range(DCH):
                        last = nc.tensor.matmul(
                            lgT[:, rg, 0 : c1 - c0],
                            wt_sb[:, a, :],
                            xb[:, a, c0:c1],
                            start=(a == 0),
                            stop=(a == DCH - 1),
                            skip_group_check=True,
                        )
                    state["last_pe"] = last
                    if rg == 0 and b > 0:
                        for jq in range(5, NJ):
                            do_square(b, jq)
                    run(rg + 1)
                    if b > 0 and rg == 2:
                        agg_chunks(b - 1, range(0, 5))
                # seg4: evict logits to fp16 on ACT; a tiny transpose warm
                # absorbs the lgnP WAR (vs the previous batch's prescale) so
                # each real transpose carries only the eviction semaphore.
                nc.scalar.copy(
                    lgsb,
                    lgT.rearrange("k r c -> k (r c)")[:, 0:N],
                )
                warmt = nc.tensor.matmul(
                    lgnP[0:2, 0, 0:2], identh[:, 0:2], identh[:, 0:2],
                    is_transpose=True, start=True, stop=True,
                    skip_group_check=True,
                )
                add_dep_helper(
                    warmt.ins, state["last_pe"].ins, sync=False,
                    reason="pin lgnP warm after prior PE work",
                )
                state["last_pe"] = warmt
                for j in range(NJ):
                    nr = NJREAL[j]
                    last = nc.tensor.matmul(
                        lgnP[0:nr, j, :],
                        lgsb[:, j * P : j * P + nr],
                        identh,
                        is_transpose=True,
                        start=True,
                        stop=True,
                        skip_group_check=True,
                    )
                state["last_pe"] = last
                if b == 0:
                    # batch 0: xt arrives only ~9us in; running the squares
                    # here instead of at the queue heads keeps ACT/DVE from
                    # stalling the kernel start (sinv runs in run(4), just
                    # before the prescale).
                    for jq in range(NJ):
                        do_square(b, jq)
                if b > 0:
                    agg_chunks(b - 1, range(5, NJ))
                run(4)
                run(5)

            emit_loads(0)
            emit_loads(1)
            for b in range(BPC):
                fillers = {}
                t = tail_pieces(b)
                if b > 0:
                    fillers[2] = [t[0]]      # sinv once ss is complete
                    fillers[4] = [t[1]]      # prescale after the transposes
                else:
                    fillers[4] = [t[0], t[1]]
                fillers[5] = [t[2], t[3], t[4], t[5]]
                if b > 1:
                    p = phase2_pieces(b - 2)
                    # negd (vl WAR) must precede the first aggregation MM
                    fillers[0] = [p[0]]
                    fillers.setdefault(4, []).insert(0, p[1])
                    fillers[5].extend([p[2], p[3]])
                phase1(b, fillers)
            # drain
            asum_mm(BPC - 2)
            p = phase2_pieces(BPC - 2)
            p[0]()
            agg_chunks(BPC - 1, range(NJ))
            asum_mm(BPC - 1)
            for f in p[1:]:
                f()
            for f in phase2_pieces(BPC - 1):
                f()

    return nc


_NC = None


def _patch_act_tables():
    """Force every ScalarE activation onto the one table set that contains
    {copy, square, ln, exp} so the kernel pays a single ACT_TABLE_LOAD
    instead of thrashing between exp_and_others and natural_log."""
    import concourse.bacc as _bacc_mod
    orig = _bacc_mod.get_activation_tables

    def patched(arch):
        tables = dict(orig(arch))
        assert "natural_log_exp_and_others" in tables
        return {
            name: (funcs if name == "natural_log_exp_and_others" else set())
            for name, funcs in tables.items()
        }

    _bacc_mod.get_activation_tables = patched


def _get_nc():
    global _NC
    if _NC is None:
        _patch_act_tables()
        nc = bacc.Bacc("TRN2", target_bir_lowering=False)
        _emit(nc)
        nc.compile()
        _NC = nc
    return _NC


def _make_in_maps(x, conv_w, centroids):
    import ml_dtypes

    bf16 = ml_dtypes.bfloat16
    B = x.shape[0]
    xp = np.zeros((B, D, NP), dtype=bf16)
    xp[:, :, 0:N] = np.asarray(x, dtype=np.float32).reshape(B, D, N).astype(bf16)
    wt = np.ascontiguousarray(np.asarray(conv_w.T, dtype=np.float32).astype(bf16))
    cent = np.ascontiguousarray(centroids, dtype=np.float32)
    in_maps = []
    for c in range(8):
        in_maps.append(
            {
                "x": np.ascontiguousarray(xp[c * BPC : (c + 1) * BPC]),
                "wt": wt,
                "cent": cent,
            }
        )
    return in_maps


def _run(x, conv_w, centroids, trace=False):
    nc = _get_nc()
    res = run_bass_kernel_spmd(
        nc,
        _make_in_maps(x, conv_w, centroids),
        core_ids=list(range(8)),
        trace=trace,
    )
    outs = [r["out"].reshape(BPC, K * D) for r in res.results]
    full = np.concatenate(outs, axis=0)
    return full, res


def kernel(x, conv_w, centroids):
    full, _ = _run(x, conv_w, centroids, trace=False)
    return full
